# revision 1
# baseline (speedup 1.0000x reference)
"""InternLM3 custom attention on 8 TRN2 NeuronCores.

Sharding: heads 4-per-core for K/V projection + attention (qk_w/v_w
column-parallel by head); AllToAll converts the attention output from
head-sharded to sequence-sharded; o-projection runs sequence-parallel
(full o_w per core) so each core emits a [256, 2048] output slice.

All matmuls run as float32r (full-rate fp32 streaming mode, free dim
kept >= 256). Attention is computed transposed (S^T[k, q]) so softmax
probabilities feed the PV matmul directly as the moving operand with
no PE transposes; the softmax denominator rides along as a ones column
appended to V. Causality: strictly-upper k-chunks are skipped at block
granularity; diagonal-band blocks are zeroed post-exp with a sliding
slice of one [128, 896] host mask.
"""

import sys

sys.path.insert(0, "/opt/trn_rl_repo")

import numpy as np

import concourse.bass as bass
import concourse.tile as tile
from concourse import bacc, mybir
from concourse.bass import ds, ts
from concourse.bass_utils import run_bass_kernel_spmd

F32 = mybir.dt.float32
F32R = mybir.dt.float32r
NCORES = 8
S = 2048          # sequence
HID = 2048        # hidden
NH = 32           # total heads
HD = 64           # head dim
HPC = NH // NCORES      # heads per core = 4
DPC = HPC * HD          # head-dims per core = 256
SSL = S // NCORES       # output seq slice per core = 256
VW = 68                 # interleaved V stride: 64 dims + 1 ones + 3 pad
ROPE_THETA = 10000.0




def build_program():
    nc = bacc.Bacc("TRN2", target_bir_lowering=False, debug=False,
                   num_devices=NCORES)

    # ---- I/O ----
    hidT = nc.dram_tensor("hidT", [HID, S], F32, kind="ExternalInput").ap()
    qkwT = nc.dram_tensor("qkwT", [HID, DPC], F32, kind="ExternalInput").ap()
    vwT = nc.dram_tensor("vwT", [HID, DPC], F32, kind="ExternalInput").ap()
    owT = nc.dram_tensor("owT", [HID, HID], F32, kind="ExternalInput").ap()
    xT_in = nc.dram_tensor("xT", [DPC, S], F32, kind="ExternalInput").ap()
    xTs_in = nc.dram_tensor("xTs", [DPC, S], F32, kind="ExternalInput").ap()
    cosT = nc.dram_tensor("cosT", [128, S], F32, kind="ExternalInput").ap()
    sinT = nc.dram_tensor("sinT", [128, S], F32, kind="ExternalInput").ap()
    maskT = nc.dram_tensor("maskT", [128, 896], F32, kind="ExternalInput").ap()
    out_sl = nc.dram_tensor("out_slice", [SSL, HID], F32,
                            kind="ExternalOutput").ap()

    with tile.TileContext(nc) as tc:
        with (
            nc.allow_low_precision(reason="float32r streaming mode, fp32 psum accum"),
            tc.tile_pool(name="const", bufs=1) as const,
            tc.tile_pool(name="dram", bufs=1, space="DRAM") as dram,
        ):
            # ---- persistent SBUF residents ----
            qkw_t = const.tile([128, 16, DPC], F32R)   # qk_w^T chunks
            nc.sync.dma_start(out=qkw_t[:],
                              in_=qkwT.rearrange("(n p) d -> p n d", p=128).bitcast(F32R))
            vw_t = const.tile([128, 16, DPC], F32R)
            nc.sync.dma_start(out=vw_t[:],
                              in_=vwT.rearrange("(n p) d -> p n d", p=128).bitcast(F32R))
            cos_t = const.tile([128, S], F32)
            nc.sync.dma_start(out=cos_t[:], in_=cosT)
            sin_t = const.tile([128, S], F32)
            nc.sync.dma_start(out=sin_t[:], in_=sinT)
            mask_t = const.tile([128, 896], F32)
            nc.sync.dma_start(out=mask_t[:], in_=maskT)
            ones_t = const.tile([1, 64], F32R)

            xt = const.tile([128, 2, S], F32R)         # X^T (2 head-pair tiles)
            nc.sync.dma_start(out=xt[:],
                              in_=xT_in.rearrange("(t p) s -> p t s", p=128).bitcast(F32R))
            kt = const.tile([128, 2, S], F32R)         # K^T, rope'd in place
            v_t = const.tile([128, 16, VW * HPC], F32R)  # V interleaved + ones
            att_t = const.tile([128, 2, S], F32)      # attn^T assembled
            # mask_t[:, 895] and row-0 cols>=384 are all 1.0 — reuse as ones
            nc.vector.tensor_copy(out=ones_t[:], in_=mask_t[0:1, 384:448])
            for h in range(HPC):
                for st in range(16):
                    nc.vector.tensor_copy(
                        out=v_t[:, st, VW * h + HD:VW * h + HD + 1],
                        in_=mask_t[:, 895:896])

            # =========== Phase A: K^T and V projections ===========
            with (
                tc.tile_pool(name="hq", bufs=6) as hpool,
                tc.tile_pool(name="psk", bufs=2, space="PSUM") as psk,
                tc.tile_pool(name="psv", bufs=4, space="PSUM") as psv,
            ):
                for sq in range(4):            # quarter of the sequence
                    pk = [psk.tile([128, 512], F32, tag='pk', name='pk') for _ in range(2)]
                    pv = [psv.tile([128, DPC], F32, tag='pv', name='pv') for _ in range(4)]
                    for hc in range(16):       # hidden-dim chunk
                        hq = hpool.tile([128, 512], F32R)
                        nc.sync.dma_start(
                            out=hq[:],
                            in_=hidT[ts(hc, 128), ts(sq, 512)].bitcast(F32R))
                        for m in range(2):
                            nc.tensor.matmul(
                                pk[m][:],
                                (qkw_t[:, hc, ts(m, 128)]),
                                (hq[:]),
                                start=(hc == 0), stop=(hc == 15))
                        for st4 in range(4):
                            nc.tensor.matmul(
                                pv[st4][:],
                                (hq[:, ts(st4, 128)]),
                                (vw_t[:, hc, :]),
                                start=(hc == 0), stop=(hc == 15))
                    for m in range(2):
                        nc.scalar.copy(out=kt[:, m, ts(sq, 512)], in_=pk[m][:])
                    for st4 in range(4):
                        for h in range(HPC):
                            nc.vector.tensor_copy(
                                out=v_t[:, sq * 4 + st4,
                                        ds(VW * h, HD)],
                                in_=pv[st4][:, ts(h, HD)])

            # =========== RoPE on X^T and K^T (in place) ===========
            with tc.tile_pool(name="sw", bufs=2) as swp:
                for t in range(2):
                    xs = swp.tile([128, S], F32, tag="sw")
                    nc.sync.dma_start(out=xs[:], in_=xTs_in[ts(t, 128), :])
                    nc.vector.tensor_mul(out=xt[:, t, :], in0=xt[:, t, :],
                                         in1=cos_t[:])
                    nc.vector.tensor_mul(out=xs[:], in0=xs[:], in1=sin_t[:])
                    nc.vector.tensor_add(out=xt[:, t, :], in0=xt[:, t, :],
                                         in1=xs[:])
                for t in range(2):
                    ks = swp.tile([128, S], F32, tag="sw")
                    # rotate_half row swap within each 64-row head block
                    for g in range(2):
                        b = 64 * g
                        nc.sync.dma_start(out=ks[b:b + 32, :],
                                          in_=kt[b + 32:b + 64, t, :].bitcast(F32))
                        nc.sync.dma_start(out=ks[b + 32:b + 64, :],
                                          in_=kt[b:b + 32, t, :].bitcast(F32))
                    nc.vector.tensor_mul(out=kt[:, t, :], in0=kt[:, t, :],
                                         in1=cos_t[:])
                    nc.vector.tensor_mul(out=ks[:], in0=ks[:], in1=sin_t[:])
                    nc.vector.tensor_add(out=kt[:, t, :], in0=kt[:, t, :],
                                         in1=ks[:])

            # =========== Phase B: attention per head ===========
            with (
                tc.tile_pool(name="pp", bufs=6) as ppool,
                tc.tile_pool(name="pss", bufs=3, space="PSUM") as pss,
                tc.tile_pool(name="pspv", bufs=2, space="PSUM") as pspv,
                tc.tile_pool(name="psbc", bufs=2, space="PSUM") as psbc,
                tc.tile_pool(name="rr", bufs=4) as rrp,
            ):
                for h in range(HPC):
                    hp = 64 * (h % 2)       # partition offset of this head
                    htl = h // 2            # which head-pair tile
                    for j in range(4):      # q block of 512
                        q0 = 512 * j
                        pvp = pspv.tile([HD + 1, 512], F32, tag='pvp')
                        nk = 4 * (j + 1)    # causal: k chunks 0..nk-1
                        for i in range(nk):
                            k0 = 128 * i
                            sp = pss.tile([128, 512], F32, tag='sp')
                            nc.tensor.matmul(
                                sp[:],
                                (kt[hp:hp + HD, htl, ts(i, 128)]),
                                (xt[hp:hp + HD, htl, ds(q0, 512)]),
                                start=True, stop=True)
                            pt = ppool.tile([128, 512], F32R, tag="pt")
                            nc.scalar.activation(
                                out=pt[:], in_=sp[:],
                                func=mybir.ActivationFunctionType.Exp,
                                scale=0.125)
                            r = k0 - q0
                            if r >= 0:      # diagonal band: causal mask
                                nc.vector.tensor_mul(
                                    out=pt[:], in0=pt[:],
                                    in1=mask_t[:, ds(384 - r, 512)])
                            nc.tensor.matmul(
                                pvp[:],
                                (v_t[:, i, ds(VW * h, HD + 1)]),
                                (pt[:]),
                                start=(i == 0), stop=(i == nk - 1))
                        # divide by denominator (row HD) & place into att_t
                        rec = rrp.tile([1, 512], F32R, tag="rec")
                        nc.vector.reciprocal(out=rec[:], in_=pvp[HD:HD + 1, :])
                        bc = psbc.tile([64, 512], F32, tag='bc')
                        nc.tensor.matmul(bc[:], (ones_t[:]),
                                         (rec[:]), start=True, stop=True)
                        nc.scalar.copy(out=att_t[hp:hp + HD, htl, ds(q0, 512)],
                                       in_=pvp[0:HD, :])
                        nc.vector.tensor_mul(
                            out=att_t[hp:hp + HD, htl, ds(q0, 512)],
                            in0=att_t[hp:hp + HD, htl, ds(q0, 512)],
                            in1=bc[:])

            # =========== Phase C: AllToAll + o-projection ===========
            a2a_in = dram.tile([NCORES, DPC, SSL], F32)
            a2a_out = dram.tile([S, SSL], F32)
            for t in range(2):
                for d in range(NCORES):
                    nc.sync.dma_start(out=a2a_in[d, ts(t, 128), :],
                                      in_=att_t[:, t, ts(d, SSL)])
            nc.gpsimd.collective_compute(
                "AllToAll",
                mybir.AluOpType.bypass,
                replica_groups=[list(range(NCORES))],
                ins=[a2a_in[:].opt()],
                outs=[a2a_out[:].opt()],
            )

            with (
                tc.tile_pool(name="af", bufs=1) as afp,
                tc.tile_pool(name="ow", bufs=4) as owp,
                tc.tile_pool(name="ob", bufs=1) as obp,
                tc.tile_pool(name="pso", bufs=8, space="PSUM") as pso,
            ):
                afull = afp.tile([128, 16, SSL], F32R)
                nc.sync.dma_start(
                    out=afull[:],
                    in_=a2a_out[:].rearrange("(n p) s -> p n s", p=128).bitcast(F32R))
                osb = obp.tile([128, 2, HID], F32)
                po = [[pso.tile([128, 512], F32, tag='po', name='po') for t in range(2)]
                      for ob in range(4)]
                for hc in range(16):
                    ow_t = owp.tile([128, HID], F32R, tag="ow")
                    nc.sync.dma_start(out=ow_t[:], in_=owT[ts(hc, 128), :].bitcast(F32R))
                    for ob in range(4):
                        for t in range(2):
                            nc.tensor.matmul(
                                po[ob][t][:],
                                (afull[:, hc, ts(t, 128)]),
                                (ow_t[:, ts(ob, 512)]),
                                start=(hc == 0), stop=(hc == 15))
                for ob in range(4):
                    for t in range(2):
                        nc.scalar.copy(out=osb[:, t, ts(ob, 512)],
                                       in_=po[ob][t][:])
                for t in range(2):
                    nc.sync.dma_start(out=out_sl[ts(t, 128), :],
                                      in_=osb[:, t, :])

    nc.compile()
    return nc


_PROGRAM = None


def _host_inputs(hidden_states, qk_w, v_w, o_w, position_ids):
    hs = np.asarray(hidden_states, dtype=np.float32)[0]          # [S, HID]
    qk_w = np.asarray(qk_w, dtype=np.float32)
    v_w = np.asarray(v_w, dtype=np.float32)
    o_w = np.asarray(o_w, dtype=np.float32)
    pos = np.asarray(position_ids)[0].astype(np.float64)         # [S]

    hidT = np.ascontiguousarray(hs.T)                            # [HID, S]
    owT = np.ascontiguousarray(o_w.T)                            # [HID, HID]

    inv_freq = 1.0 / (ROPE_THETA ** (np.arange(0, HD, 2, dtype=np.float64) / HD))
    freqs = pos[None, :] * inv_freq[:, None]                     # [32, S]
    emb = np.concatenate([freqs, freqs], axis=0)                 # [64, S]
    cos1 = np.cos(emb).astype(np.float32)
    sin1 = np.sin(emb).astype(np.float32)
    sin_signed = sin1.copy()
    sin_signed[:HD // 2] *= -1.0                                 # fold rotate sign
    cosT = np.tile(cos1, (2, 1)).astype(np.float32)              # [128, S]
    sinT = np.tile(sin_signed, (2, 1)).astype(np.float32)

    kl = np.arange(128)[:, None]
    u = np.arange(896)[None, :]
    maskT = (u >= kl + 384).astype(np.float32)                   # [128, 896]

    in_maps = []
    for c in range(NCORES):
        rows = slice(DPC * c, DPC * (c + 1))
        xT = hidT[rows]                                          # [256, S]
        xTs = np.empty_like(xT)                                  # rotate_half rows
        for h in range(HPC):
            b = HD * h
            xTs[b:b + 32] = xT[b + 32:b + 64]
            xTs[b + 32:b + 64] = xT[b:b + 32]
        in_maps.append({
            "hidT": hidT,
            "qkwT": np.ascontiguousarray(qk_w[rows].T),          # [HID, 256]
            "vwT": np.ascontiguousarray(v_w[rows].T),
            "owT": owT,
            "xT": np.ascontiguousarray(xT),
            "xTs": np.ascontiguousarray(xTs),
            "cosT": cosT,
            "sinT": sinT,
            "maskT": maskT,
        })
    return in_maps


def kernel(hidden_states, qk_w, v_w, o_w, position_ids, **extra):
    global _PROGRAM
    if _PROGRAM is None:
        _PROGRAM = build_program()
    in_maps = _host_inputs(hidden_states, qk_w, v_w, o_w, position_ids)
    res = run_bass_kernel_spmd(_PROGRAM, in_maps, list(range(NCORES)))
    out = np.concatenate([res.results[c]["out_slice"]
                          for c in range(NCORES)], axis=0)
    return out.reshape(1, S, HID).astype(np.float32)



# revision 3
# speedup vs baseline: 2.4934x; 2.4934x over previous
"""InternLM3 custom attention on 8 TRN2 NeuronCores.

Sharding: heads 4-per-core for K/V projection + attention (qk_w/v_w
column-parallel by head); AllToAll converts the attention output from
head-sharded to sequence-sharded; o-projection runs sequence-parallel
(full o_w per core) so each core emits a [256, 2048] output slice.

All matmuls run as float32r (full-rate fp32 streaming mode, free dim
kept >= 256). Attention is computed transposed (S^T[k, q]) so softmax
probabilities feed the PV matmul directly as the moving operand with
no PE transposes; the softmax denominator rides along as a ones column
appended to V. Causality: strictly-upper k-chunks are skipped at block
granularity; diagonal-band blocks are zeroed post-exp with a sliding
slice of one [128, 896] host mask.
"""

import sys

sys.path.insert(0, "/opt/trn_rl_repo")

import numpy as np

import concourse.bass as bass
import concourse.tile as tile
from concourse import bacc, mybir
from concourse.bass import ds, ts
from concourse.bass_utils import run_bass_kernel_spmd

F32 = mybir.dt.float32
F32R = mybir.dt.float32r
NCORES = 8
S = 2048          # sequence
HID = 2048        # hidden
NH = 32           # total heads
HD = 64           # head dim
HPC = NH // NCORES      # heads per core = 4
DPC = HPC * HD          # head-dims per core = 256
SSL = S // NCORES       # output seq slice per core = 256
VW = 68                 # interleaved V stride: 64 dims + 1 ones + 3 pad
ROPE_THETA = 10000.0




def build_program(sim_no_collective=False):
    nc = bacc.Bacc("TRN2", target_bir_lowering=False, debug=False,
                   num_devices=NCORES)

    # ---- I/O ----
    hidT = nc.dram_tensor("hidT", [HID, S], F32, kind="ExternalInput").ap()
    qkwT = nc.dram_tensor("qkwT", [HID, DPC], F32, kind="ExternalInput").ap()
    vwT = nc.dram_tensor("vwT", [HID, DPC], F32, kind="ExternalInput").ap()
    owT = nc.dram_tensor("owT", [HID, HID], F32, kind="ExternalInput").ap()
    xT_in = nc.dram_tensor("xT", [DPC, S], F32, kind="ExternalInput").ap()
    xTs_in = nc.dram_tensor("xTs", [DPC, S], F32, kind="ExternalInput").ap()
    cosT = nc.dram_tensor("cosT", [128, S], F32, kind="ExternalInput").ap()
    sinT = nc.dram_tensor("sinT", [128, S], F32, kind="ExternalInput").ap()
    maskT = nc.dram_tensor("maskT", [128, 896], F32, kind="ExternalInput").ap()
    out_sl = nc.dram_tensor("out_slice", [SSL, HID], F32,
                            kind="ExternalOutput").ap()

    with tile.TileContext(nc) as tc:
        with (
            nc.allow_low_precision(reason="float32r streaming mode, fp32 psum accum"),
            tc.tile_pool(name="const", bufs=1) as const,
            tc.tile_pool(name="dram", bufs=1, space="DRAM") as dram,
        ):
            # ---- persistent SBUF residents ----
            qkw_t = const.tile([128, 16, DPC], F32R)   # qk_w^T chunks
            nc.sync.dma_start(out=qkw_t[:],
                              in_=qkwT.rearrange("(n p) d -> p n d", p=128).bitcast(F32R))
            vw_t = const.tile([128, 16, DPC], F32R)
            nc.sync.dma_start(out=vw_t[:],
                              in_=vwT.rearrange("(n p) d -> p n d", p=128).bitcast(F32R))
            cos_t = const.tile([128, S], F32)
            nc.sync.dma_start(out=cos_t[:], in_=cosT)
            sin_t = const.tile([128, S], F32)
            nc.sync.dma_start(out=sin_t[:], in_=sinT)
            mask_t = const.tile([128, 896], F32)
            nc.sync.dma_start(out=mask_t[:], in_=maskT)
            ones_t = const.tile([1, 64], F32R)

            xt = const.tile([128, 2, S], F32R)         # X^T (2 head-pair tiles)
            nc.sync.dma_start(out=xt[:],
                              in_=xT_in.rearrange("(t p) s -> p t s", p=128).bitcast(F32R))
            kt = const.tile([128, 2, S], F32R)         # K^T, rope'd in place
            v_t = const.tile([128, 16, VW * HPC], F32R)  # V interleaved + ones
            att_t = const.tile([128, 2, S], F32)      # attn^T assembled
            # mask_t[:, 895] and row-0 cols>=384 are all 1.0 — reuse as ones
            nc.vector.tensor_copy(out=ones_t[:], in_=mask_t[0:1, 384:448])
            for h in range(HPC):
                for st in range(16):
                    nc.vector.tensor_copy(
                        out=v_t[:, st, VW * h + HD:VW * h + HD + 1],
                        in_=mask_t[:, 895:896])

            # =========== Phase A: K^T and V projections ===========
            with (
                tc.tile_pool(name="hq", bufs=6) as hpool,
                tc.tile_pool(name="psk", bufs=2, space="PSUM") as psk,
                tc.tile_pool(name="psv", bufs=4, space="PSUM") as psv,
            ):
                for sq in range(4):            # quarter of the sequence
                    pk = [psk.tile([128, 512], F32, tag='pk', name='pk') for _ in range(2)]
                    pv = [psv.tile([128, DPC], F32, tag='pv', name='pv') for _ in range(4)]
                    for hc in range(16):       # hidden-dim chunk
                        hq = hpool.tile([128, 512], F32R)
                        nc.sync.dma_start(
                            out=hq[:],
                            in_=hidT[ts(hc, 128), ts(sq, 512)].bitcast(F32R))
                        for m in range(2):
                            nc.tensor.matmul(
                                pk[m][:],
                                (qkw_t[:, hc, ts(m, 128)]),
                                (hq[:]),
                                start=(hc == 0), stop=(hc == 15))
                        for st4 in range(4):
                            nc.tensor.matmul(
                                pv[st4][:],
                                (hq[:, ts(st4, 128)]),
                                (vw_t[:, hc, :]),
                                start=(hc == 0), stop=(hc == 15))
                    for m in range(2):
                        nc.scalar.copy(out=kt[:, m, ts(sq, 512)], in_=pk[m][:])
                    for st4 in range(4):
                        for h in range(HPC):
                            nc.vector.tensor_copy(
                                out=v_t[:, sq * 4 + st4,
                                        ds(VW * h, HD)],
                                in_=pv[st4][:, ts(h, HD)])

            # =========== RoPE on X^T and K^T (in place) ===========
            with tc.tile_pool(name="sw", bufs=2) as swp:
                for t in range(2):
                    xs = swp.tile([128, S], F32, tag="sw")
                    nc.sync.dma_start(out=xs[:], in_=xTs_in[ts(t, 128), :])
                    nc.vector.tensor_mul(out=xt[:, t, :], in0=xt[:, t, :],
                                         in1=cos_t[:])
                    nc.vector.tensor_mul(out=xs[:], in0=xs[:], in1=sin_t[:])
                    nc.vector.tensor_add(out=xt[:, t, :], in0=xt[:, t, :],
                                         in1=xs[:])
                for t in range(2):
                    ks = swp.tile([128, S], F32, tag="sw")
                    # rotate_half row swap within each 64-row head block
                    for g in range(2):
                        b = 64 * g
                        nc.sync.dma_start(out=ks[b:b + 32, :],
                                          in_=kt[b + 32:b + 64, t, :].bitcast(F32))
                        nc.sync.dma_start(out=ks[b + 32:b + 64, :],
                                          in_=kt[b:b + 32, t, :].bitcast(F32))
                    nc.vector.tensor_mul(out=kt[:, t, :], in0=kt[:, t, :],
                                         in1=cos_t[:])
                    nc.vector.tensor_mul(out=ks[:], in0=ks[:], in1=sin_t[:])
                    nc.vector.tensor_add(out=kt[:, t, :], in0=kt[:, t, :],
                                         in1=ks[:])

            # =========== Phase B: attention per head ===========
            with (
                tc.tile_pool(name="pp", bufs=6) as ppool,
                tc.tile_pool(name="pss", bufs=3, space="PSUM") as pss,
                tc.tile_pool(name="pspv", bufs=2, space="PSUM") as pspv,
                tc.tile_pool(name="psbc", bufs=2, space="PSUM") as psbc,
                tc.tile_pool(name="rr", bufs=4) as rrp,
            ):
                for h in range(HPC):
                    hp = 64 * (h % 2)       # partition offset of this head
                    htl = h // 2            # which head-pair tile
                    for j in range(4):      # q block of 512
                        q0 = 512 * j
                        pvp = pspv.tile([HD + 1, 512], F32, tag='pvp')
                        nk = 4 * (j + 1)    # causal: k chunks 0..nk-1
                        for i in range(nk):
                            k0 = 128 * i
                            sp = pss.tile([128, 512], F32, tag='sp')
                            nc.tensor.matmul(
                                sp[:],
                                (kt[hp:hp + HD, htl, ts(i, 128)]),
                                (xt[hp:hp + HD, htl, ds(q0, 512)]),
                                start=True, stop=True)
                            pt = ppool.tile([128, 512], F32R, tag="pt")
                            nc.scalar.activation(
                                out=pt[:], in_=sp[:],
                                func=mybir.ActivationFunctionType.Exp,
                                scale=0.125)
                            r = k0 - q0
                            if r >= 0:      # diagonal band: causal mask
                                nc.vector.tensor_mul(
                                    out=pt[:], in0=pt[:],
                                    in1=mask_t[:, ds(384 - r, 512)])
                            nc.tensor.matmul(
                                pvp[:],
                                (v_t[:, i, ds(VW * h, HD + 1)]),
                                (pt[:]),
                                start=(i == 0), stop=(i == nk - 1))
                        # divide by denominator (row HD) & place into att_t
                        rec = rrp.tile([1, 512], F32R, tag="rec")
                        nc.vector.reciprocal(out=rec[:], in_=pvp[HD:HD + 1, :])
                        bc = psbc.tile([64, 512], F32, tag='bc')
                        nc.tensor.matmul(bc[:], (ones_t[:]),
                                         (rec[:]), start=True, stop=True)
                        nc.scalar.copy(out=att_t[hp:hp + HD, htl, ds(q0, 512)],
                                       in_=pvp[0:HD, :])
                        nc.vector.tensor_mul(
                            out=att_t[hp:hp + HD, htl, ds(q0, 512)],
                            in0=att_t[hp:hp + HD, htl, ds(q0, 512)],
                            in1=bc[:])

            # =========== Phase C: AllToAll + o-projection ===========
            a2a_in = dram.tile([NCORES, DPC, SSL], F32)
            a2a_out = dram.tile([S, SSL], F32)
            for t in range(2):
                for d in range(NCORES):
                    nc.sync.dma_start(out=a2a_in[d, ts(t, 128), :],
                                      in_=att_t[:, t, ts(d, SSL)])
            if sim_no_collective:
                nc.sync.dma_start(
                    out=a2a_out[:],
                    in_=a2a_in[:].rearrange("d p s -> (d p) s"))
            else:
                nc.gpsimd.collective_compute(
                    "AllToAll",
                    mybir.AluOpType.bypass,
                    replica_groups=[list(range(NCORES))],
                    ins=[a2a_in[:].opt()],
                    outs=[a2a_out[:].opt()],
                )

            with (
                tc.tile_pool(name="af", bufs=1) as afp,
                tc.tile_pool(name="ow", bufs=4) as owp,
                tc.tile_pool(name="ob", bufs=1) as obp,
                tc.tile_pool(name="pso", bufs=8, space="PSUM") as pso,
            ):
                afull = afp.tile([128, 16, SSL], F32R)
                nc.sync.dma_start(
                    out=afull[:],
                    in_=a2a_out[:].rearrange("(n p) s -> p n s", p=128).bitcast(F32R))
                osb = obp.tile([128, 2, HID], F32)
                po = [[pso.tile([128, 512], F32, tag='po', name='po') for t in range(2)]
                      for ob in range(4)]
                for hc in range(16):
                    ow_t = owp.tile([128, HID], F32R, tag="ow")
                    nc.sync.dma_start(out=ow_t[:], in_=owT[ts(hc, 128), :].bitcast(F32R))
                    for ob in range(4):
                        for t in range(2):
                            nc.tensor.matmul(
                                po[ob][t][:],
                                (afull[:, hc, ts(t, 128)]),
                                (ow_t[:, ts(ob, 512)]),
                                start=(hc == 0), stop=(hc == 15))
                for ob in range(4):
                    for t in range(2):
                        nc.scalar.copy(out=osb[:, t, ts(ob, 512)],
                                       in_=po[ob][t][:])
                for t in range(2):
                    nc.sync.dma_start(out=out_sl[ts(t, 128), :],
                                      in_=osb[:, t, :])

    nc.compile()
    return nc


_PROGRAM = None


def _host_inputs(hidden_states, qk_w, v_w, o_w, position_ids):
    hs = np.asarray(hidden_states, dtype=np.float32)[0]          # [S, HID]
    qk_w = np.asarray(qk_w, dtype=np.float32)
    v_w = np.asarray(v_w, dtype=np.float32)
    o_w = np.asarray(o_w, dtype=np.float32)
    pos = np.asarray(position_ids)[0].astype(np.float64)         # [S]

    hidT = np.ascontiguousarray(hs.T)                            # [HID, S]
    owT = np.ascontiguousarray(o_w.T)                            # [HID, HID]

    inv_freq = 1.0 / (ROPE_THETA ** (np.arange(0, HD, 2, dtype=np.float64) / HD))
    freqs = pos[None, :] * inv_freq[:, None]                     # [32, S]
    emb = np.concatenate([freqs, freqs], axis=0)                 # [64, S]
    cos1 = np.cos(emb).astype(np.float32)
    sin1 = np.sin(emb).astype(np.float32)
    sin_signed = sin1.copy()
    sin_signed[:HD // 2] *= -1.0                                 # fold rotate sign
    cosT = np.tile(cos1, (2, 1)).astype(np.float32)              # [128, S]
    sinT = np.tile(sin_signed, (2, 1)).astype(np.float32)

    kl = np.arange(128)[:, None]
    u = np.arange(896)[None, :]
    maskT = (u >= kl + 384).astype(np.float32)                   # [128, 896]

    in_maps = []
    for c in range(NCORES):
        rows = slice(DPC * c, DPC * (c + 1))
        xT = hidT[rows]                                          # [256, S]
        xTs = np.empty_like(xT)                                  # rotate_half rows
        for h in range(HPC):
            b = HD * h
            xTs[b:b + 32] = xT[b + 32:b + 64]
            xTs[b + 32:b + 64] = xT[b:b + 32]
        in_maps.append({
            "hidT": hidT,
            "qkwT": np.ascontiguousarray(qk_w[rows].T),          # [HID, 256]
            "vwT": np.ascontiguousarray(v_w[rows].T),
            "owT": owT,
            "xT": np.ascontiguousarray(xT),
            "xTs": np.ascontiguousarray(xTs),
            "cosT": cosT,
            "sinT": sinT,
            "maskT": maskT,
        })
    return in_maps


def kernel(hidden_states, qk_w, v_w, o_w, position_ids, **extra):
    global _PROGRAM
    if _PROGRAM is None:
        _PROGRAM = build_program()
    in_maps = _host_inputs(hidden_states, qk_w, v_w, o_w, position_ids)
    res = run_bass_kernel_spmd(_PROGRAM, in_maps, list(range(NCORES)))
    out = np.concatenate([res.results[c]["out_slice"]
                          for c in range(NCORES)], axis=0)
    return out.reshape(1, S, HID).astype(np.float32)

